# revision 51
# baseline (speedup 1.0000x reference)
"""CrystalGraphEncoder (2x TransformerConv + 2x GATConv + LN + mean-pool + MLP)
as a Bass/Tile kernel on 8 Trainium2 NeuronCores.

Strategy: shard destination nodes across cores (edges sorted by dst). Per
layer: sharded dense matmuls -> fp8 src-row tables (kv for TransformerConv /
hh for GAT) -> AllGather in two halves into Shared-HBM tables (split by
source node so the second half overlaps edge compute) -> bulk dma_gather of
per-edge src rows (round-robin over the 4 SWDGE queues so descriptor
generation spreads over all Q7 core pairs) -> dst-side per-edge values (q
rows / GAT a_d) are NOT gathered: they are expanded on-chip with one-hot
S^T matmuls from SBUF-resident per-block projections -> DVE edge math ->
one-hot (fp8) scatter matmuls into PSUM -> pair-batched normalize + skip +
residual + LN on-chip. The dense phase of layer L+1 is interleaved into the
edge phase of layer L so its AllGather halves are emitted as early as their
bounce rows are ready. Mean-pool via one-hot matmul + AllReduce; final MLP
replicated.
"""
import numpy as np
import ml_dtypes

import concourse.bacc as bacc
import concourse.tile as tile
from concourse import bass, mybir
from concourse import bass_utils
from concourse.masks import make_identity

F16 = mybir.dt.float16
F32 = mybir.dt.float32
F8 = mybir.dt.float8e4
I16 = mybir.dt.int16
NP_F16 = np.float16
NP_F8 = ml_dtypes.float8_e4m3
P = 128

# problem constants (from the reference model)
N_NODES = 20000
IN_DIM = 92
HID = 256
OUT_DIM = 128
HEADS = 8
HDIM = 32
N_GRAPHS = 128
LN_EPS = 1e-5
C = 8  # cores
AF = mybir.ActivationFunctionType


def _wrap_idxs(idx):
    """[n] int -> [128, n//16] int16 dma_gather index layout (16-partition wrap,
    replicated for the 8 Q7 cores)."""
    n = len(idx)
    assert n % 16 == 0
    w = idx.reshape(n // 16, 16).T.astype(np.int16)
    return np.ascontiguousarray(np.tile(w, (8, 1)))


def _edge_struct(src_half_row, src_half, dst_local, dst_core, NB):
    """Per-core gather/scatter arrays for one edge set, split by source half
    so the AllGather can be chunked into two overlappable halves.

    Per dst block: slots are [NTA j-tiles of half-0-src edges | NTB j-tiles of
    half-1-src edges]. kv idx arrays are stored per half (block-major) so a
    block-pair's half gather is one contiguous slice. S is the scatter
    one-hot (edge slot p -> dst slot d); ST is its per-tile transpose
    (dst slot d -> edge slot p) used to expand dst-side values (q / a_d)
    to edge slots via matmul instead of a per-edge gather.
    """
    blk = dst_local // P
    slot = dst_local % P
    key = (dst_core * NB + blk) * 2 + src_half
    order = np.argsort(key, kind="stable")
    src_s = src_half_row[order]
    slot_s = slot[order]
    counts = np.bincount(key, minlength=C * NB * 2)
    NTA = int(np.ceil(counts[0::2].max() / P))
    NTB = int(np.ceil(counts[1::2].max() / P))
    NT = NTA + NTB
    starts = np.concatenate([[0], np.cumsum(counts)])

    per_core = []
    for c in range(C):
        kvA = np.zeros((NB, NTA * P), dtype=np.int64)  # pad -> row 0
        kvB = np.zeros((NB, NTB * P), dtype=np.int64)
        S = np.zeros((NB, P, NT * P), dtype=NP_F8)
        ST = np.zeros((NB, P, NT * P), dtype=NP_F8)
        for b in range(NB):
            for half, kv_arr, base in ((0, kvA, 0), (1, kvB, NTA * P)):
                k = (c * NB + b) * 2 + half
                s, e = starts[k], starts[k + 1]
                n = e - s
                pos = np.arange(n)
                kv_arr[b, pos] = src_s[s:e]
                jj = pos // P
                pp = pos % P
                col = base + jj * P
                S[b, pp, col + slot_s[s:e]] = 1.0
                ST[b, slot_s[s:e], col + pp] = 1.0
        per_core.append(
            dict(
                kv_idxA=_wrap_idxs(kvA.reshape(-1)),
                kv_idxB=_wrap_idxs(kvB.reshape(-1)),
                S=np.ascontiguousarray(S.reshape(NB * P, NT * P)),
                ST=np.ascontiguousarray(ST.reshape(NB * P, NT * P)),
            )
        )
    return NTA, NTB, per_core


def host_prep(inputs):
    """Split + pad + sort everything on the host. Returns (meta, in_maps)."""
    x = np.asarray(inputs["x"], np.float32)
    ei = np.asarray(inputs["edge_index"], np.int64)
    batch = np.asarray(inputs["batch"], np.int64)
    N = x.shape[0]
    RPC = (N + C - 1) // C
    NB = (RPC + P - 1) // P
    NPC = NB * P

    core_of = np.minimum(np.arange(N) // RPC, C - 1)
    local_of = np.arange(N) - core_of * RPC
    HNPC = NPC // 2
    half_of = local_of // HNPC  # which AllGather half this node's row is in
    # row within the half table [C * HNPC, width]
    ghrow = core_of * HNPC + (local_of % HNPC)

    src, dst = ei[0], ei[1]
    NTTA, NTTB, tconv = _edge_struct(
        ghrow[src], half_of[src], local_of[dst], core_of[dst], NB
    )
    sl = np.arange(N, dtype=np.int64)
    src_g = np.concatenate([src, sl])
    dst_g = np.concatenate([dst, sl])
    NTGA, NTGB, gat = _edge_struct(
        ghrow[src_g], half_of[src_g], local_of[dst_g], core_of[dst_g], NB
    )

    cnt = np.bincount(batch, minlength=N_GRAPHS).astype(np.float32)
    invcnt = (1.0 / np.maximum(cnt, 1.0)).reshape(N_GRAPHS, 1)

    def f16(a):
        return np.asarray(a, np.float32).astype(NP_F16)

    def pack_k(w):  # [K, N] -> [128, K//128 * N] (chunk-major)
        w = np.asarray(w, np.float32)
        K, Nc = w.shape
        assert K % P == 0
        return np.ascontiguousarray(
            w.reshape(K // P, P, Nc).transpose(1, 0, 2).reshape(P, -1)
        ).astype(NP_F16)

    wdict = dict(
        win=f16(inputs["Win"]),
        b_in=f16(np.asarray(inputs["b_in"]).reshape(1, HID)),
        w1=pack_k(inputs["W1"]),
        b1=f16(np.asarray(inputs["b1"]).reshape(1, 2 * HID)),
        w2=pack_k(inputs["W2"]),
        b2=f16(np.asarray(inputs["b2"]).reshape(1, OUT_DIM)),
        invcnt=invcnt.astype(np.float32),
    )
    for t in range(2):
        wkv = np.concatenate(
            [np.asarray(inputs["Wk"][t]), np.asarray(inputs["Wv"][t])], axis=1
        )
        bkv = np.concatenate(
            [np.asarray(inputs["bk"][t]), np.asarray(inputs["bv"][t])]
        )
        wdict[f"wkv{t}"] = pack_k(wkv)
        wdict[f"bkv{t}"] = f16(bkv.reshape(1, 2 * HID))
        wdict[f"wq{t}"] = pack_k(inputs["Wq"][t])
        wdict[f"bq{t}"] = f16(np.asarray(inputs["bq"][t]).reshape(1, HID))
        wdict[f"wsk{t}"] = pack_k(
            np.asarray(inputs["Wskip"][t], np.float64) + np.eye(HID)
        )
        wdict[f"bsk{t}"] = f16(np.asarray(inputs["bskip"][t]).reshape(1, HID))
        wdict[f"wg{t}"] = pack_k(inputs["Wg"][t])
        wdict[f"bg{t}"] = f16(np.asarray(inputs["bg"][t]).reshape(1, HID))
        wdict[f"atts{t}"] = np.ascontiguousarray(
            np.broadcast_to(
                f16(np.asarray(inputs["att_src"][t]).reshape(1, HID)), (P, HID)
            )
        )
        wdict[f"attd{t}"] = np.ascontiguousarray(
            np.broadcast_to(
                f16(np.asarray(inputs["att_dst"][t]).reshape(1, HID)), (P, HID)
            )
        )

    ln_g = np.asarray(inputs["ln_g"], np.float32)
    ln_b = np.asarray(inputs["ln_b"], np.float32)
    ln_trivial = bool(np.all(ln_g == 1.0) and np.all(ln_b == 0.0))
    if not ln_trivial:
        for i in range(4):
            wdict[f"lng{i}"] = np.ascontiguousarray(
                np.broadcast_to(ln_g[i].reshape(1, HID).astype(NP_F16), (P, HID))
            )
            wdict[f"lnb{i}"] = np.ascontiguousarray(
                np.broadcast_to(ln_b[i].reshape(1, HID).astype(NP_F16), (P, HID))
            )

    in_maps = []
    for c in range(C):
        m = dict(wdict)
        lo, hi = c * RPC, min((c + 1) * RPC, N)
        xT = np.zeros((IN_DIM, NPC), np.float32)
        xT[:, 0 : hi - lo] = x[lo:hi].T
        m["xT"] = xT.astype(NP_F16)
        m["kvidxA"] = tconv[c]["kv_idxA"]
        m["kvidxB"] = tconv[c]["kv_idxB"]
        m["S_t"] = tconv[c]["S"]
        m["ST_t"] = tconv[c]["ST"]
        m["gatidxA"] = gat[c]["kv_idxA"]
        m["gatidxB"] = gat[c]["kv_idxB"]
        m["S_g"] = gat[c]["S"]
        m["ST_g"] = gat[c]["ST"]
        Sp = np.zeros((NB, P, N_GRAPHS), dtype=NP_F8)
        ns = hi - lo
        bb = np.arange(ns) // P
        pp = np.arange(ns) % P
        Sp[bb, pp, batch[lo:hi]] = 1.0
        m["S_p"] = np.ascontiguousarray(Sp.reshape(NB * P, N_GRAPHS))
        in_maps.append(m)

    meta = dict(
        NB=NB, NPC=NPC, NTTA=NTTA, NTTB=NTTB, NTGA=NTGA, NTGB=NTGB,
        ln_trivial=ln_trivial,
    )
    return meta, in_maps


def build_program(meta, reps=1, skip=frozenset()):
    NB = meta["NB"]
    NPC = meta["NPC"]
    NTTA, NTTB = meta["NTTA"], meta["NTTB"]
    NTGA, NTGB = meta["NTGA"], meta["NTGB"]
    ln_trivial = meta["ln_trivial"]
    HNPC = NPC // 2
    HTABN = C * HNPC
    NTT = NTTA + NTTB  # j-tiles per block
    NTG = NTGA + NTGB
    NTMAX = max(NTT, NTG)
    GATW = 256  # f16 cols: 0:128 hh as fp8 (bitcast), 128:136 a_s, rest zero

    nc = bacc.Bacc(
        "TRN2", target_bir_lowering=False, debug=False, num_devices=C,
        num_swdge_queues=4, dynamic_dma_scratch_size=14336,
    )
    _gq_counter = [0]

    def _next_gq():
        q = _gq_counter[0] % 4
        _gq_counter[0] += 1
        return q

    def di(name, shape, dt):
        return nc.dram_tensor(name, shape, dt, kind="ExternalInput")

    xT_d = di("xT", [IN_DIM, NPC], F16)
    kvidxA_d = di("kvidxA", [P, NB * NTTA * 8], I16)
    kvidxB_d = di("kvidxB", [P, NB * NTTB * 8], I16)
    St_d = di("S_t", [NB * P, NTT * P], F8)
    STt_d = di("ST_t", [NB * P, NTT * P], F8)
    gatidxA_d = di("gatidxA", [P, NB * NTGA * 8], I16)
    gatidxB_d = di("gatidxB", [P, NB * NTGB * 8], I16)
    Sg_d = di("S_g", [NB * P, NTG * P], F8)
    STg_d = di("ST_g", [NB * P, NTG * P], F8)
    Sp_d = di("S_p", [NB * P, N_GRAPHS], F8)
    invcnt_d = di("invcnt", [N_GRAPHS, 1], F32)
    win_d = di("win", [IN_DIM, HID], F16)
    bin_d = di("b_in", [1, HID], F16)
    w1_d = di("w1", [P, 2 * 2 * HID], F16)
    b1_d = di("b1", [1, 2 * HID], F16)
    w2_d = di("w2", [P, 4 * OUT_DIM], F16)
    b2_d = di("b2", [1, OUT_DIM], F16)
    wd = {}
    for t in range(2):
        wd[f"wkv{t}"] = di(f"wkv{t}", [P, 2 * 2 * HID], F16)
        wd[f"bkv{t}"] = di(f"bkv{t}", [1, 2 * HID], F16)
        wd[f"wq{t}"] = di(f"wq{t}", [P, 2 * HID], F16)
        wd[f"bq{t}"] = di(f"bq{t}", [1, HID], F16)
        wd[f"wsk{t}"] = di(f"wsk{t}", [P, 2 * HID], F16)
        wd[f"bsk{t}"] = di(f"bsk{t}", [1, HID], F16)
        wd[f"wg{t}"] = di(f"wg{t}", [P, 2 * HID], F16)
        wd[f"bg{t}"] = di(f"bg{t}", [1, HID], F16)
        wd[f"atts{t}"] = di(f"atts{t}", [P, HID], F16)
        wd[f"attd{t}"] = di(f"attd{t}", [P, HID], F16)
    if not ln_trivial:
        for i in range(4):
            wd[f"lng{i}"] = di(f"lng{i}", [P, HID], F16)
            wd[f"lnb{i}"] = di(f"lnb{i}", [P, HID], F16)

    out_d = nc.dram_tensor("out", [N_GRAPHS, OUT_DIM], F32, kind="ExternalOutput")

    h_all = nc.alloc_sbuf_tensor("h_all", [P, NB * HID], F16)
    hT_all = nc.alloc_sbuf_tensor("hT_all", [P, 2 * NPC], F8)
    # per-block dst-side projections kept on-chip (expanded to edge slots by
    # one-hot S^T matmuls -- never gathered)
    q8_all = nc.alloc_sbuf_tensor("q8_all", [P, NB * HID], F8)
    ad_all = nc.alloc_sbuf_tensor("ad_all", [P, NB * HEADS], F16)

    SQ32 = 1.0 / float(np.sqrt(HDIM))

    with tile.TileContext(nc) as tc:
        with (
            tc.tile_pool(name="wpool", bufs=1) as wp,
            tc.tile_pool(name="spool", bufs=2) as sp,
            tc.tile_pool(name="gpool", bufs=2) as gp,
            tc.tile_pool(name="psA", bufs=1, space="PSUM") as psA,
            tc.tile_pool(name="psB", bufs=1, space="PSUM") as psB,
            tc.tile_pool(name="psG", bufs=2, space="PSUM") as psG,
            tc.tile_pool(name="psQ", bufs=2, space="PSUM") as psQ,
            tc.tile_pool(name="psT", bufs=1, space="PSUM") as psT,
            tc.tile_pool(name="psC", bufs=1, space="PSUM") as psC,
            tc.tile_pool(name="dram", bufs=1, space="DRAM") as dp,
        ):
            ident = wp.tile([P, P], F16, tag="ident")
            make_identity(nc, ident[:])
            ones1 = wp.tile([1, P], F16, tag="ones1")
            nc.vector.memset(ones1[:], 1.0)
            eps_t = wp.tile([P, 1], F32, tag="eps")
            nc.vector.memset(eps_t[:], LN_EPS)

            def load_w(d, shape, tag, dt=F16):
                t = wp.tile(shape, dt, tag=tag)
                nc.sync.dma_start(t[:], d.ap())
                return t

            win_t = load_w(win_d, [IN_DIM, HID], "win")
            bin_t = load_w(bin_d, [1, HID], "b_in")
            w1_t = load_w(w1_d, [P, 2 * 2 * HID], "w1")
            b1_t = load_w(b1_d, [1, 2 * HID], "b1")
            w2_t = load_w(w2_d, [P, 4 * OUT_DIM], "w2")
            b2_t = load_w(b2_d, [1, OUT_DIM], "b2")
            wt = {}
            for t in range(2):
                for nm, sh in [
                    (f"wkv{t}", [P, 2 * 2 * HID]),
                    (f"bkv{t}", [1, 2 * HID]),
                    (f"wq{t}", [P, 2 * HID]),
                    (f"bq{t}", [1, HID]),
                    (f"wsk{t}", [P, 2 * HID]),
                    (f"bsk{t}", [1, HID]),
                    (f"wg{t}", [P, 2 * HID]),
                    (f"bg{t}", [1, HID]),
                    (f"atts{t}", [P, HID]),
                    (f"attd{t}", [P, HID]),
                ]:
                    wt[nm] = load_w(wd[nm], sh, nm)
            if not ln_trivial:
                for i in range(4):
                    wt[f"lng{i}"] = load_w(wd[f"lng{i}"], [P, HID], f"lng{i}")
                    wt[f"lnb{i}"] = load_w(wd[f"lnb{i}"], [P, HID], f"lnb{i}")
            invcnt_t = load_w(invcnt_d, [N_GRAPHS, 1], "invcnt", F32)

            def mm_dense(psum, lhsT0, lhsT1, w_tile, ncols, bias_tile):
                nc.tensor.matmul(
                    psum, lhsT=lhsT0, rhs=w_tile[:, 0:ncols], start=True, stop=False
                )
                nc.tensor.matmul(
                    psum, lhsT=lhsT1, rhs=w_tile[:, ncols : 2 * ncols],
                    start=False, stop=False,
                )
                nc.tensor.matmul(
                    psum, lhsT=ones1[:], rhs=bias_tile[:, 0:ncols],
                    start=False, stop=True,
                )

            def hT_slices(b):
                l0 = hT_all.ap()[:, 0 * NPC + b * P : 0 * NPC + (b + 1) * P]
                l1 = hT_all.ap()[:, 1 * NPC + b * P : 1 * NPC + (b + 1) * P]
                return l0, l1

            def store_hT(b):
                """h_all (f16) block b -> transposed fp8 copies in hT_all.
                Both 128-col transposes land in one PSUM tile; a single
                strided ACT copy writes both hT_all chunks."""
                hsl = h_all.ap()[:, b * HID : (b + 1) * HID]
                ptp = psT.tile([P, 2, P], F16, space="PSUM", tag="ptp")
                for f in range(2):
                    nc.tensor.transpose(
                        ptp[:, f, :], hsl[:, f * P : (f + 1) * P], ident[:]
                    )
                nc.scalar.activation(
                    hT_all.ap().rearrange("p (f c) -> p f c", f=2)[
                        :, :, b * P : (b + 1) * P
                    ],
                    ptp[:],
                    AF.Copy,
                )

            for _rep in range(reps):
                # phase 0: h0 = x @ Win + b_in
                for b in range(NB):
                    xt_b = sp.tile([IN_DIM, P], F16, tag="xT_b")
                    nc.sync.dma_start(xt_b[:], xT_d.ap()[:, b * P : (b + 1) * P])
                    ps = psA.tile([P, 2 * HID], F32, space="PSUM", tag="ps_dense")
                    nc.tensor.matmul(
                        ps[:, 0:HID], lhsT=xt_b[:],
                        rhs=win_t[:], start=True, stop=False,
                    )
                    nc.tensor.matmul(
                        ps[:, 0:HID], lhsT=ones1[:], rhs=bin_t[:], start=False,
                        stop=True,
                    )
                    nc.scalar.activation(
                        h_all.ap()[:, b * HID : (b + 1) * HID], ps[:, 0:HID], AF.Copy
                    )
                    store_hT(b)

                # ---- tables: per-layer, per-half fp8 AllGather outputs ----
                # Each AllGather half is a separate Shared tensor (single-writer
                # rule) so the second half can fly while edges of the first half
                # are being processed. GAT rows are hh-only fp8 (a_s is
                # recomputed per edge on-chip).
                kv_tabs = [
                    [
                        dp.tile(
                            [HTABN, 2 * HID], F8, tag=f"kv_tab{t}{h}_r{_rep}",
                            name=f"kv_tab{t}{h}_r{_rep}", addr_space="Shared",
                        )
                        for h in range(2)
                    ]
                    for t in range(2)
                ]
                gat_tabs = [
                    [
                        dp.tile(
                            [HTABN, GATW], F16, tag=f"gat_tab{t}{h}_r{_rep}",
                            name=f"gat_tab{t}{h}_r{_rep}", addr_space="Shared",
                        )
                        for h in range(2)
                    ]
                    for t in range(2)
                ]
                kv_bnc = dp.tile([NPC, 2 * HID], F8, tag=f"kv_bnc_r{_rep}")
                gat_bnc = dp.tile([NPC, GATW], F16, tag=f"gat_bnc_r{_rep}")
                # zero the unread tail columns once so the AllGather input is
                # fully initialized
                zpad = wp.tile([P, GATW], F16, tag="zpad")
                nc.vector.memset(zpad[:], 0.0)
                for b in range(NB):
                    nc.sync.dma_start(
                        gat_bnc[b * P : (b + 1) * P, HID // 2 + HEADS : GATW],
                        zpad[:, 0 : GATW - HID // 2 - HEADS],
                    )
                pool_in = dp.tile([N_GRAPHS, HID], F32, tag=f"pool_in_r{_rep}")
                pool_out = dp.tile([N_GRAPHS, HID], F32, tag=f"pool_out_r{_rep}")

                psum_pool = psC.tile([N_GRAPHS, HID], F32, space="PSUM", tag="ps_pool")
                agg_sb_q = sp.tile([P, 4, HID + HEADS], F16, tag="agg_sb")
                skp_sb_q = sp.tile([P, 4, HID], F16, tag="skp_sb")
                quad = {"agg": agg_sb_q, "skp": skp_sb_q, "blocks": []}

                def dense_pair(layer, bp):
                    """Dense projections for blocks (bp, bp+1) of `layer` into
                    the bounce buffers / on-chip q8/ad tables."""
                    if "dense" in skip:
                        return
                    is_t = layer % 2 == 0
                    t = layer // 2
                    nb2 = min(2, NB - bp)
                    prow = (
                        lambda tab, c0, c1: tab[bp * P : (bp + nb2) * P, c0:c1]
                        .rearrange("(i p) f -> p i f", i=nb2)
                    )
                    if is_t:
                        kv8p = sp.tile([P, 2, 2 * HID], F8, tag="kv8")
                    else:
                        hh8p = sp.tile([P, 2, HID], F8, tag="hh8")
                        as16p = sp.tile([P, 2, HEADS], F16, tag="as16")
                    for i in range(nb2):
                        b = bp + i
                        l0, l1 = hT_slices(b)
                        if is_t:
                            ps = psA.tile(
                                [P, 2 * HID], F32, space="PSUM", tag="ps_dense"
                            )
                            mm_dense(
                                ps[:], l0, l1, wt[f"wkv{t}"], 2 * HID, wt[f"bkv{t}"]
                            )
                            nc.scalar.activation(kv8p[:, i, :], ps[:], AF.Copy)
                            ps2 = psB.tile([P, 2 * HID], F32, space="PSUM", tag="ps_b")
                            mm_dense(
                                ps2[:, 0:HID], l0, l1, wt[f"wq{t}"], HID, wt[f"bq{t}"]
                            )
                            nc.scalar.activation(
                                q8_all.ap()[:, b * HID : (b + 1) * HID],
                                ps2[:, 0:HID], AF.Copy,
                            )
                        else:
                            ps = psA.tile(
                                [P, 2 * HID], F32, space="PSUM", tag="ps_dense"
                            )
                            mm_dense(
                                ps[:, 0:HID], l0, l1, wt[f"wg{t}"], HID, wt[f"bg{t}"]
                            )
                            nc.scalar.activation(hh8p[:, i, :], ps[:, 0:HID], AF.Copy)
                            for which, wnm in ((0, f"atts{t}"), (1, f"attd{t}")):
                                proda = sp.tile([P, HID], F16, tag="prodA")
                                nc.vector.tensor_tensor(
                                    out=proda[:], in0=ps[:, 0:HID], in1=wt[wnm][:],
                                    op=mybir.AluOpType.mult,
                                )
                                asum = sp.tile([P, HEADS], F32, tag="asum")
                                nc.vector.tensor_reduce(
                                    out=asum[:],
                                    in_=proda[:].rearrange(
                                        "p (h w) -> p h w", h=HEADS
                                    ),
                                    axis=mybir.AxisListType.X,
                                    op=mybir.AluOpType.add,
                                )
                                dst = (
                                    as16p[:, i, :]
                                    if which == 0
                                    else ad_all.ap()[:, b * HEADS : (b + 1) * HEADS]
                                )
                                nc.scalar.activation(dst, asum[:], AF.Copy)
                    if is_t:
                        nc.sync.dma_start(prow(kv_bnc, 0, 2 * HID), kv8p[:, 0:nb2, :])
                    else:
                        nc.sync.dma_start(
                            prow(gat_bnc, 0, HID // 2), hh8p[:, 0:nb2, :].bitcast(F16)
                        )
                        nc.sync.dma_start(
                            prow(gat_bnc, HID // 2, HID // 2 + HEADS),
                            as16p[:, 0:nb2, :],
                        )

                def emit_ag(layer, half):
                    """AllGather one half of this layer's table."""
                    if "ag" in skip:
                        return
                    is_t = layer % 2 == 0
                    t = layer // 2
                    rows = slice(half * HNPC, (half + 1) * HNPC)
                    if is_t:
                        nc.gpsimd.collective_compute(
                            "AllGather",
                            mybir.AluOpType.bypass,
                            replica_groups=[list(range(C))],
                            ins=[kv_bnc[rows, :]],
                            outs=[kv_tabs[t][half][:]],
                        )
                    else:
                        nc.gpsimd.collective_compute(
                            "AllGather",
                            mybir.AluOpType.bypass,
                            replica_groups=[list(range(C))],
                            ins=[gat_bnc[rows, :]],
                            outs=[gat_tabs[t][half][:]],
                        )

                def edge_pair(layer, bp):
                    if "edge" in skip:
                        return
                    do_edvec = "edvec" not in skip
                    do_scatter = "scatter" not in skip
                    do_norm = do_scatter and "norm" not in skip
                    is_t = layer % 2 == 0
                    t = layer // 2
                    nb2 = min(2, NB - bp)
                    NTa = NTTA if is_t else NTGA
                    NTb = NTTB if is_t else NTGB
                    NT = NTa + NTb
                    TT = nb2 * NT
                    N2a = nb2 * NTa
                    islA = slice(bp * NTa * 8, (bp + nb2) * NTa * 8)
                    islB = slice(bp * NTb * 8, (bp + nb2) * NTb * 8)
                    tabs = kv_tabs[t] if is_t else gat_tabs[t]
                    S_d = St_d if is_t else Sg_d
                    ST_d = STt_d if is_t else STg_d

                    def tmap(tg):
                        """g_kv tile index -> (block-in-pair, block-local tile)."""
                        if tg < N2a:
                            return tg // NTa, tg % NTa
                        tg -= N2a
                        return tg // NTb, NTa + tg % NTb

                    if is_t:
                        g_kv = gp.tile([P, 2 * NT, 2 * HID], F8, tag="g_big1")
                        ROWW = 2 * HID
                        idxA_d, idxB_d = kvidxA_d, kvidxB_d
                    else:
                        g_kv = gp.tile([P, 2 * NT, GATW], F16, tag="g_big0")
                        ROWW = GATW
                        idxA_d, idxB_d = gatidxA_d, gatidxB_d
                    idxA_t = gp.tile(
                        [P, 2 * max(NTTA, NTGA) * 8], I16, tag="idxA"
                    )
                    idxB_t = gp.tile(
                        [P, 2 * max(NTTB, NTGB) * 8], I16, tag="idxB"
                    )
                    nc.sync.dma_start(
                        idxA_t[:, 0 : nb2 * NTa * 8], idxA_d.ap()[:, islA]
                    )
                    nc.sync.dma_start(
                        idxB_t[:, 0 : nb2 * NTb * 8], idxB_d.ap()[:, islB]
                    )
                    # tiles per gather call: must fit the SWDGE ring
                    # (896 descs); 7 for GAT avoids a 2-tile runt call
                    GCH = 6 if is_t else 7
                    if "gather" not in skip:
                        for t0 in range(0, N2a, GCH):
                            tn = min(GCH, N2a - t0)
                            nc.gpsimd.dma_gather(
                                g_kv[:, t0 : t0 + tn, :], tabs[0][:],
                                idxA_t[:, t0 * 8 : (t0 + tn) * 8],
                                tn * P, tn * P, ROWW, single_packet=False,
                                queue_num=_next_gq(),
                            )
                        nbt = nb2 * NTb
                        for t0 in range(0, nbt, GCH):
                            tn = min(GCH, nbt - t0)
                            nc.gpsimd.dma_gather(
                                g_kv[:, N2a + t0 : N2a + t0 + tn, :], tabs[1][:],
                                idxB_t[:, t0 * 8 : (t0 + tn) * 8],
                                tn * P, tn * P, ROWW, single_packet=False,
                                queue_num=_next_gq(),
                            )
                    if is_t:
                        vpart = g_kv[:, 0:TT, HID : 2 * HID]
                    else:
                        vpart = g_kv[:, 0:TT, 0 : HID // 2].bitcast(F8)

                    ST_sb = []
                    for i in range(nb2):
                        b = bp + i
                        stt = gp.tile([P, NTMAX * P], F8, tag=f"ST{i}")
                        nc.sync.dma_start(
                            stt[:, 0 : NT * P], ST_d.ap()[b * P : (b + 1) * P, :]
                        )
                        ST_sb.append(stt)

                    rhs = gp.tile([P, 2 * NTMAX, HID + HEADS], F16, tag="rhs")
                    red = gp.tile([P, 2 * NTMAX * HEADS], F16, tag="red")
                    expdst = rhs[:, 0:TT, HID : HID + HEADS]
                    if not do_edvec:
                        pass
                    elif is_t:
                        # q[dst] expanded per edge slot: psq[:, g, :] =
                        # ST_tile^T @ q8_block, then k * q product on DVE.
                        for pt in range((TT + 1) // 2):
                            ng = min(2, TT - 2 * pt)
                            psq = psQ.tile([P, 2, HID], F32, space="PSUM", tag="psq")
                            for g in range(ng):
                                i, tl = tmap(2 * pt + g)
                                nc.tensor.matmul(
                                    psq[:, g, :],
                                    lhsT=ST_sb[i][:, tl * P : (tl + 1) * P],
                                    rhs=q8_all.ap()[
                                        :, (bp + i) * HID : (bp + i + 1) * HID
                                    ],
                                    start=True, stop=True,
                                )
                            nc.vector.tensor_tensor(
                                out=rhs[:, 2 * pt : 2 * pt + ng, 0:HID],
                                in0=g_kv[:, 2 * pt : 2 * pt + ng, 0:HID],
                                in1=psq[:, 0:ng, :],
                                op=mybir.AluOpType.mult,
                            )
                        with nc.allow_low_precision(reason="f16 logits"):
                            nc.vector.tensor_reduce(
                                out=red[:, 0 : TT * HEADS],
                                in_=rhs[:, 0:TT, 0:HID].rearrange(
                                    "p t (h w) -> p t h w", h=HEADS
                                ),
                                axis=mybir.AxisListType.X,
                                op=mybir.AluOpType.add,
                            )
                        nc.scalar.activation(
                            expdst,
                            red[:, 0 : TT * HEADS].rearrange(
                                "p (t h) -> p t h", h=HEADS
                            ),
                            AF.Exp,
                            scale=SQ32,
                        )
                    else:
                        # a_d[dst] expanded per edge slot into PSUM (8-col
                        # one-hot matmuls); a_s recomputed from gathered hh.
                        psq = psQ.tile([P, 2, HID], F32, space="PSUM", tag="psq")
                        psad = (
                            psq[:]
                            .rearrange("p a b -> p (a b)")[:, 0 : TT * HEADS]
                            .rearrange("p (t h) -> p t h", h=HEADS)
                        )
                        for tg in range(TT):
                            i, tl = tmap(tg)
                            nc.tensor.matmul(
                                psad[:, tg, :],
                                lhsT=ST_sb[i][:, tl * P : (tl + 1) * P],
                                rhs=ad_all.ap()[
                                    :, (bp + i) * HEADS : (bp + i + 1) * HEADS
                                ],
                                start=True, stop=True,
                            )
                        esum = gp.tile([P, 2 * NTMAX * HEADS], F16, tag="esum")
                        nc.vector.tensor_tensor(
                            out=esum[:, 0 : TT * HEADS].rearrange(
                                "p (t h) -> p t h", h=HEADS
                            ),
                            in0=g_kv[:, 0:TT, HID // 2 : HID // 2 + HEADS],
                            in1=psad[:, 0:TT, :],
                            op=mybir.AluOpType.add,
                        )
                        # leaky_relu(x, 0.2) = 0.6x + 0.4|x| (expdst doubles
                        # as |x| scratch; Exp overwrites it right after)
                        esum3 = esum[:, 0 : TT * HEADS].rearrange(
                            "p (t h) -> p t h", h=HEADS
                        )
                        nc.scalar.activation(expdst, esum3, AF.Abs, scale=0.4)
                        nc.vector.scalar_tensor_tensor(
                            out=red[:, 0 : TT * HEADS].rearrange(
                                "p (t h) -> p t h", h=HEADS
                            ),
                            in0=esum3,
                            scalar=0.6,
                            in1=expdst,
                            op0=mybir.AluOpType.mult,
                            op1=mybir.AluOpType.add,
                        )
                        nc.scalar.activation(
                            expdst,
                            red[:, 0 : TT * HEADS].rearrange(
                                "p (t h) -> p t h", h=HEADS
                            ),
                            AF.Exp,
                        )
                    if do_edvec:
                        nc.vector.tensor_tensor(
                            out=rhs[:, 0:TT, 0:HID].rearrange(
                                "p t (h w) -> p t h w", h=HEADS
                            ),
                            in0=vpart.rearrange("p t (h w) -> p t h w", h=HEADS),
                            in1=expdst.to_broadcast([P, TT, HEADS, HDIM]),
                            op=mybir.AluOpType.mult,
                        )
                    # per-block scatter + skip matmuls
                    aggs = []
                    for i in range(nb2 if do_scatter else 0):
                        b = bp + i
                        # reuse the ST tile: its one-hot data has been fully
                        # consumed by the psq/psad matmuls by this point
                        S_sb = ST_sb[i]
                        nc.sync.dma_start(
                            S_sb[:, 0 : NT * P], S_d.ap()[b * P : (b + 1) * P, :]
                        )
                        ps_agg = psG.tile(
                            [P, HID + HEADS], F32, space="PSUM", tag="ps_agg"
                        )
                        for jj in range(NTa):
                            nc.tensor.matmul(
                                ps_agg[:],
                                lhsT=S_sb[:, jj * P : (jj + 1) * P],
                                rhs=rhs[:, i * NTa + jj, :],
                                start=(jj == 0),
                                stop=False,
                            )
                        for jj in range(NTb):
                            nc.tensor.matmul(
                                ps_agg[:],
                                lhsT=S_sb[:, (NTa + jj) * P : (NTa + jj + 1) * P],
                                rhs=rhs[:, N2a + i * NTb + jj, :],
                                start=False,
                                stop=(jj == NTb - 1),
                            )
                        l0, l1 = hT_slices(b)
                        ps_skip = psB.tile([P, 2 * HID], F32, space="PSUM", tag="ps_b")
                        if is_t:
                            mm_dense(
                                ps_skip[:, 0:HID], l0, l1, wt[f"wsk{t}"], HID,
                                wt[f"bsk{t}"],
                            )
                        else:
                            nc.tensor.matmul(
                                ps_skip[:, 0:HID], lhsT=ones1[:], rhs=wt[f"bg{t}"][:],
                                start=True, stop=True,
                            )
                        aggs.append((ps_agg, ps_skip))

                    # stage PSUM accumulators to SBUF on ACT; the DVE
                    # normalize chain runs once per TWO pairs (4 blocks) in
                    # norm_quad so its op count halves.
                    if not do_norm:
                        return
                    q = (bp // 2) % 2  # slot within the quad staging buffer
                    agg_sb = quad["agg"]
                    skp_sb = quad["skp"]
                    for i in range(nb2):
                        ps_agg, ps_skip = aggs[i]
                        nc.scalar.activation(
                            agg_sb[:, 2 * q + i, :], ps_agg[:], AF.Copy
                        )
                        nc.scalar.activation(
                            skp_sb[:, 2 * q + i, :], ps_skip[:, 0:HID], AF.Copy
                        )
                    quad["blocks"] += list(range(bp, bp + nb2))

                def norm_quad(layer):
                    """Normalize + LN + relu for the staged quad of blocks."""
                    is_t = layer % 2 == 0
                    blocks = quad["blocks"]
                    nb2 = len(blocks)
                    if nb2 == 0:
                        return
                    quad["blocks"] = []
                    agg_sb = quad["agg"]
                    skp_sb = quad["skp"]
                    bp = blocks[0]
                    t2p = sp.tile([P, 4, HID], F16, tag="t2p")
                    den = sp.tile([P, 4, HEADS], F32, tag="den")
                    nc.vector.tensor_scalar(
                        out=den[:, 0:nb2, :],
                        in0=agg_sb[:, 0:nb2, HID : HID + HEADS],
                        scalar1=1e-16, scalar2=None, op0=mybir.AluOpType.add,
                    )
                    rec = sp.tile([P, 4, HEADS], F32, tag="rec")
                    nc.vector.reciprocal(rec[:, 0:nb2, :], den[:, 0:nb2, :])
                    nc.vector.tensor_tensor(
                        out=t2p[:, 0:nb2, :].rearrange(
                            "p i (h w) -> p i h w", h=HEADS
                        ),
                        in0=agg_sb[:, 0:nb2, 0:HID].rearrange(
                            "p i (h w) -> p i h w", h=HEADS
                        ),
                        in1=rec[:, 0:nb2, :].to_broadcast([P, nb2, HEADS, HDIM]),
                        op=mybir.AluOpType.mult,
                    )
                    nc.vector.tensor_tensor(
                        out=t2p[:, 0:nb2, :], in0=t2p[:, 0:nb2, :],
                        in1=skp_sb[:, 0:nb2, :],
                        op=mybir.AluOpType.add,
                    )
                    t2v = t2p[:, 0:nb2, :]
                    if not is_t:
                        nc.vector.tensor_tensor(
                            out=t2v, in0=t2v,
                            in1=h_all.ap()[:, bp * HID : (bp + nb2) * HID].rearrange(
                                "p (i f) -> p i f", i=nb2
                            ),
                            op=mybir.AluOpType.add,
                        )
                    assert blocks == list(range(bp, bp + nb2))
                    mu = sp.tile([P, 4], F32, tag="mu")
                    nc.vector.tensor_reduce(
                        out=mu[:, 0:nb2], in_=t2v, axis=mybir.AxisListType.X,
                        op=mybir.AluOpType.add,
                    )
                    nc.vector.tensor_scalar(
                        out=mu[:, 0:nb2], in0=mu[:, 0:nb2], scalar1=1.0 / HID,
                        scalar2=None, op0=mybir.AluOpType.mult,
                    )
                    nc.vector.tensor_tensor(
                        out=t2v, in0=t2v,
                        in1=mu[:, 0:nb2].to_broadcast([P, nb2, HID]),
                        op=mybir.AluOpType.subtract,
                    )
                    sq = sp.tile([P, 4, HID], F16, tag="sq")
                    nc.scalar.activation(sq[:, 0:nb2, :], t2v, AF.Square)
                    s2 = sp.tile([P, 4], F32, tag="s2")
                    nc.vector.tensor_reduce(
                        out=s2[:, 0:nb2], in_=sq[:, 0:nb2, :],
                        axis=mybir.AxisListType.X, op=mybir.AluOpType.add,
                    )
                    # rsqrt via exp(-0.5*ln(x)): Sqrt lives in an ACT
                    # table set without Exp, so Sqrt/Exp alternation would
                    # reload the ACT function table each pair.
                    sd = sp.tile([P, 4], F32, tag="sd")
                    nc.scalar.activation(
                        sd[:, 0:nb2], s2[:, 0:nb2], AF.Ln, scale=1.0 / HID,
                        bias=eps_t[:, 0:1],
                    )
                    rs = sp.tile([P, 4], F32, tag="rs")
                    nc.scalar.activation(
                        rs[:, 0:nb2], sd[:, 0:nb2], AF.Exp, scale=-0.5
                    )
                    nc.vector.tensor_tensor(
                        out=t2v, in0=t2v,
                        in1=rs[:, 0:nb2].to_broadcast([P, nb2, HID]),
                        op=mybir.AluOpType.mult,
                    )
                    if not ln_trivial:
                        nc.vector.tensor_tensor(
                            out=t2v, in0=t2v,
                            in1=wt[f"lng{layer}"][:]
                            .rearrange("p (o f) -> p o f", o=1)
                            .to_broadcast([P, nb2, HID]),
                            op=mybir.AluOpType.mult,
                        )
                        nc.vector.tensor_tensor(
                            out=t2v, in0=t2v,
                            in1=wt[f"lnb{layer}"][:]
                            .rearrange("p (o f) -> p o f", o=1)
                            .to_broadcast([P, nb2, HID]),
                            op=mybir.AluOpType.add,
                        )
                    hdst = h_all.ap()[:, bp * HID : (bp + nb2) * HID]
                    nc.vector.tensor_scalar(
                        out=hdst.rearrange("p (i f) -> p i f", i=nb2), in0=t2v,
                        scalar1=0.0, scalar2=None, op0=mybir.AluOpType.max,
                    )
                    for i in range(nb2):
                        b = bp + i
                        if "sth" not in skip:
                            store_hT(b)
                        if layer == 3:
                            spt = sp.tile([P, N_GRAPHS], F8, tag="Sp_b")
                            nc.sync.dma_start(
                                spt[:], Sp_d.ap()[b * P : (b + 1) * P, :]
                            )
                            nc.tensor.matmul(
                                psum_pool[:],
                                lhsT=spt[:],
                                rhs=h_all.ap()[:, b * HID : (b + 1) * HID],
                                start=(b == 0),
                                stop=(b == NB - 1),
                            )

                # layer 0 dense phase + its chunked AllGathers
                for bp in range(0, NB, 2):
                    dense_pair(0, bp)
                    if bp == NB // 2 - 2:
                        emit_ag(0, 0)
                emit_ag(0, 1)

                # main loop: edge phase of layer L interleaved with dense phase
                # of layer L+1; each half-AllGather is emitted as soon as its
                # bounce rows are complete so it overlaps remaining edge/dense
                # work.
                for layer in range(4):
                    for bp in range(0, NB, 2):
                        edge_pair(layer, bp)
                        if (bp // 2) % 2 == 1 or bp + 2 >= NB:
                            norm_quad(layer)
                            agg_sb_q = sp.tile(
                                [P, 4, HID + HEADS], F16, tag="agg_sb"
                            )
                            skp_sb_q = sp.tile([P, 4, HID], F16, tag="skp_sb")
                            quad["agg"] = agg_sb_q
                            quad["skp"] = skp_sb_q
                            if layer < 3:
                                for bq in range(max(0, bp - 2), bp + 2, 2):
                                    dense_pair(layer + 1, bq)
                                    if bq == NB // 2 - 2:
                                        emit_ag(layer + 1, 0)
                                    elif bq == NB - 2:
                                        emit_ag(layer + 1, 1)

                # ---- pool + MLP ----
                if skip & {"edge", "scatter", "norm"}:  # ablation: pool unwritten
                    nc.tensor.matmul(
                        psum_pool[:], lhsT=ident[:], rhs=h_all.ap()[:, 0:HID],
                        start=True, stop=True,
                    )
                pool_sb = sp.tile([N_GRAPHS, HID], F32, tag="pool_sb")
                nc.scalar.activation(pool_sb[:], psum_pool[:], AF.Copy)
                nc.sync.dma_start(pool_in[:], pool_sb[:])
                if "ar" not in skip:
                    nc.gpsimd.collective_compute(
                        "AllReduce",
                        mybir.AluOpType.add,
                        replica_groups=[list(range(C))],
                        ins=[pool_in.opt()],
                        outs=[pool_out.opt()],
                    )
                sums = sp.tile([N_GRAPHS, HID], F32, tag="sums")
                nc.sync.dma_start(
                    sums[:], pool_in[:] if "ar" in skip else pool_out[:]
                )
                pooled = sp.tile([N_GRAPHS, HID], F32, tag="pooled")
                nc.vector.tensor_scalar(
                    out=pooled[:], in0=sums[:], scalar1=invcnt_t[:, 0:1],
                    scalar2=None, op0=mybir.AluOpType.mult,
                )
                p16 = sp.tile([N_GRAPHS, HID], F16, tag="p16")
                nc.scalar.activation(p16[:], pooled[:], AF.Copy)
                pT = sp.tile([P, 2 * N_GRAPHS], F16, tag="pT")
                for f in range(2):
                    ptp = psT.tile([P, P], F16, space="PSUM", tag="ptp")
                    nc.tensor.transpose(ptp[:], p16[:, f * P : (f + 1) * P], ident[:])
                    nc.scalar.activation(
                        pT[:, f * N_GRAPHS : (f + 1) * N_GRAPHS], ptp[:], AF.Copy
                    )
                ps1 = psA.tile([P, 2 * HID], F32, space="PSUM", tag="ps_dense")
                nc.tensor.matmul(
                    ps1[:], lhsT=pT[:, 0:N_GRAPHS], rhs=w1_t[:, 0 : 2 * HID],
                    start=True, stop=False,
                )
                nc.tensor.matmul(
                    ps1[:], lhsT=pT[:, N_GRAPHS : 2 * N_GRAPHS],
                    rhs=w1_t[:, 2 * HID : 4 * HID], start=False, stop=False,
                )
                nc.tensor.matmul(
                    ps1[:], lhsT=ones1[:], rhs=b1_t[:], start=False, stop=True
                )
                h1 = sp.tile([N_GRAPHS, 2 * HID], F16, tag="h1")
                nc.scalar.activation(h1[:], ps1[:], AF.Relu)
                h1T = sp.tile([P, 4 * N_GRAPHS], F16, tag="h1T")
                for f in range(4):
                    ptp = psT.tile([P, P], F16, space="PSUM", tag="ptp")
                    nc.tensor.transpose(ptp[:], h1[:, f * P : (f + 1) * P], ident[:])
                    nc.scalar.activation(
                        h1T[:, f * N_GRAPHS : (f + 1) * N_GRAPHS], ptp[:], AF.Copy
                    )
                ps2 = psB.tile([P, 2 * HID], F32, space="PSUM", tag="ps_b")
                for f in range(4):
                    nc.tensor.matmul(
                        ps2[:, 0:OUT_DIM],
                        lhsT=h1T[:, f * N_GRAPHS : (f + 1) * N_GRAPHS],
                        rhs=w2_t[:, f * OUT_DIM : (f + 1) * OUT_DIM],
                        start=(f == 0),
                        stop=False,
                    )
                nc.tensor.matmul(
                    ps2[:, 0:OUT_DIM], lhsT=ones1[:], rhs=b2_t[:], start=False,
                    stop=True,
                )
                out_sb = sp.tile([N_GRAPHS, OUT_DIM], F32, tag="out_sb")
                nc.scalar.activation(out_sb[:], ps2[:, 0:OUT_DIM], AF.Copy)
                nc.sync.dma_start(out_d.ap(), out_sb[:])

    nc.compile()
    return nc


_CACHE = {}


def kernel(**inputs):
    meta, in_maps = host_prep(inputs)
    key = tuple(sorted(meta.items()))
    if key not in _CACHE:
        _CACHE[key] = build_program(meta)
    nc = _CACHE[key]
    res = bass_utils.run_bass_kernel_spmd(nc, in_maps, core_ids=list(range(C)))
    return np.asarray(res.results[0]["out"], np.float32)


# revision 53
# speedup vs baseline: 1.1040x; 1.1040x over previous
"""CrystalGraphEncoder (2x TransformerConv + 2x GATConv + LN + mean-pool + MLP)
as a Bass/Tile kernel on 8 Trainium2 NeuronCores.

Strategy: shard destination nodes across cores (edges sorted by dst). Per
layer: sharded dense matmuls -> fp8 src-row tables (kv for TransformerConv /
hh for GAT) -> AllGather in two halves into Shared-HBM tables (split by
source node so the second half overlaps edge compute) -> bulk dma_gather of
per-edge src rows (round-robin over the 4 SWDGE queues so descriptor
generation spreads over all Q7 core pairs) -> dst-side per-edge values (q
rows / GAT a_d) are NOT gathered: they are expanded on-chip with one-hot
S^T matmuls from SBUF-resident per-block projections -> DVE edge math ->
one-hot (fp8) scatter matmuls into PSUM -> pair-batched normalize + skip +
residual + LN on-chip. The dense phase of layer L+1 is interleaved into the
edge phase of layer L so its AllGather halves are emitted as early as their
bounce rows are ready. Mean-pool via one-hot matmul + AllReduce; final MLP
replicated.
"""
import numpy as np
import ml_dtypes

import concourse.bacc as bacc
import concourse.tile as tile
from concourse import bass, mybir
from concourse import bass_utils
from concourse.masks import make_identity

F16 = mybir.dt.float16
F32 = mybir.dt.float32
F8 = mybir.dt.float8e4
I16 = mybir.dt.int16
NP_F16 = np.float16
NP_F8 = ml_dtypes.float8_e4m3
P = 128

# problem constants (from the reference model)
N_NODES = 20000
IN_DIM = 92
HID = 256
OUT_DIM = 128
HEADS = 8
HDIM = 32
N_GRAPHS = 128
LN_EPS = 1e-5
C = 8  # cores
AF = mybir.ActivationFunctionType


def _wrap_idxs(idx):
    """[n] int -> [128, n//16] int16 dma_gather index layout (16-partition wrap,
    replicated for the 8 Q7 cores)."""
    n = len(idx)
    assert n % 16 == 0
    w = idx.reshape(n // 16, 16).T.astype(np.int16)
    return np.ascontiguousarray(np.tile(w, (8, 1)))


def _edge_struct(src_half_row, src_half, dst_local, dst_core, NB):
    """Per-core gather/scatter arrays for one edge set, split by source half
    so the AllGather can be chunked into two overlappable halves.

    Per dst block: slots are [NTA j-tiles of half-0-src edges | NTB j-tiles of
    half-1-src edges]. kv idx arrays are stored per half (block-major) so a
    block-pair's half gather is one contiguous slice. S is the scatter
    one-hot (edge slot p -> dst slot d); ST is its per-tile transpose
    (dst slot d -> edge slot p) used to expand dst-side values (q / a_d)
    to edge slots via matmul instead of a per-edge gather.
    """
    blk = dst_local // P
    slot = dst_local % P
    key = (dst_core * NB + blk) * 2 + src_half
    order = np.argsort(key, kind="stable")
    src_s = src_half_row[order]
    slot_s = slot[order]
    counts = np.bincount(key, minlength=C * NB * 2)
    NTA = int(np.ceil(counts[0::2].max() / P))
    NTB = int(np.ceil(counts[1::2].max() / P))
    NT = NTA + NTB
    starts = np.concatenate([[0], np.cumsum(counts)])

    per_core = []
    for c in range(C):
        kvA = np.zeros((NB, NTA * P), dtype=np.int64)  # pad -> row 0
        kvB = np.zeros((NB, NTB * P), dtype=np.int64)
        S = np.zeros((NB, P, NT * P), dtype=NP_F8)
        ST = np.zeros((NB, P, NT * P), dtype=NP_F8)
        for b in range(NB):
            for half, kv_arr, base in ((0, kvA, 0), (1, kvB, NTA * P)):
                k = (c * NB + b) * 2 + half
                s, e = starts[k], starts[k + 1]
                n = e - s
                pos = np.arange(n)
                kv_arr[b, pos] = src_s[s:e]
                jj = pos // P
                pp = pos % P
                col = base + jj * P
                S[b, pp, col + slot_s[s:e]] = 1.0
                ST[b, slot_s[s:e], col + pp] = 1.0
        per_core.append(
            dict(
                kv_idxA=_wrap_idxs(kvA.reshape(-1)),
                kv_idxB=_wrap_idxs(kvB.reshape(-1)),
                S=np.ascontiguousarray(S.reshape(NB * P, NT * P)),
                ST=np.ascontiguousarray(ST.reshape(NB * P, NT * P)),
            )
        )
    return NTA, NTB, per_core


def host_prep(inputs):
    """Split + pad + sort everything on the host. Returns (meta, in_maps)."""
    x = np.asarray(inputs["x"], np.float32)
    ei = np.asarray(inputs["edge_index"], np.int64)
    batch = np.asarray(inputs["batch"], np.int64)
    N = x.shape[0]
    RPC = (N + C - 1) // C
    NB = (RPC + P - 1) // P
    NPC = NB * P

    core_of = np.minimum(np.arange(N) // RPC, C - 1)
    local_of = np.arange(N) - core_of * RPC
    HNPC = NPC // 2
    half_of = local_of // HNPC  # which AllGather half this node's row is in
    # row within the half table [C * HNPC, width]
    ghrow = core_of * HNPC + (local_of % HNPC)

    src, dst = ei[0], ei[1]
    NTTA, NTTB, tconv = _edge_struct(
        ghrow[src], half_of[src], local_of[dst], core_of[dst], NB
    )
    sl = np.arange(N, dtype=np.int64)
    src_g = np.concatenate([src, sl])
    dst_g = np.concatenate([dst, sl])
    NTGA, NTGB, gat = _edge_struct(
        ghrow[src_g], half_of[src_g], local_of[dst_g], core_of[dst_g], NB
    )

    cnt = np.bincount(batch, minlength=N_GRAPHS).astype(np.float32)
    invcnt = (1.0 / np.maximum(cnt, 1.0)).reshape(N_GRAPHS, 1)

    def f16(a):
        return np.asarray(a, np.float32).astype(NP_F16)

    def pack_k(w):  # [K, N] -> [128, K//128 * N] (chunk-major)
        w = np.asarray(w, np.float32)
        K, Nc = w.shape
        assert K % P == 0
        return np.ascontiguousarray(
            w.reshape(K // P, P, Nc).transpose(1, 0, 2).reshape(P, -1)
        ).astype(NP_F16)

    wdict = dict(
        win=f16(inputs["Win"]),
        b_in=f16(np.asarray(inputs["b_in"]).reshape(1, HID)),
        w1=pack_k(inputs["W1"]),
        b1=f16(np.asarray(inputs["b1"]).reshape(1, 2 * HID)),
        w2=pack_k(inputs["W2"]),
        b2=f16(np.asarray(inputs["b2"]).reshape(1, OUT_DIM)),
        invcnt=invcnt.astype(np.float32),
    )
    for t in range(2):
        wkv = np.concatenate(
            [np.asarray(inputs["Wk"][t]), np.asarray(inputs["Wv"][t])], axis=1
        )
        bkv = np.concatenate(
            [np.asarray(inputs["bk"][t]), np.asarray(inputs["bv"][t])]
        )
        wdict[f"wkv{t}"] = pack_k(wkv)
        wdict[f"bkv{t}"] = f16(bkv.reshape(1, 2 * HID))
        wdict[f"wq{t}"] = pack_k(inputs["Wq"][t])
        wdict[f"bq{t}"] = f16(np.asarray(inputs["bq"][t]).reshape(1, HID))
        wdict[f"wsk{t}"] = pack_k(
            np.asarray(inputs["Wskip"][t], np.float64) + np.eye(HID)
        )
        wdict[f"bsk{t}"] = f16(np.asarray(inputs["bskip"][t]).reshape(1, HID))
        wdict[f"wg{t}"] = pack_k(inputs["Wg"][t])
        wdict[f"bg{t}"] = f16(np.asarray(inputs["bg"][t]).reshape(1, HID))
        wdict[f"atts{t}"] = np.ascontiguousarray(
            np.broadcast_to(
                f16(np.asarray(inputs["att_src"][t]).reshape(1, HID)), (P, HID)
            )
        )
        wdict[f"attd{t}"] = np.ascontiguousarray(
            np.broadcast_to(
                f16(np.asarray(inputs["att_dst"][t]).reshape(1, HID)), (P, HID)
            )
        )

    ln_g = np.asarray(inputs["ln_g"], np.float32)
    ln_b = np.asarray(inputs["ln_b"], np.float32)
    ln_trivial = bool(np.all(ln_g == 1.0) and np.all(ln_b == 0.0))
    if not ln_trivial:
        for i in range(4):
            wdict[f"lng{i}"] = np.ascontiguousarray(
                np.broadcast_to(ln_g[i].reshape(1, HID).astype(NP_F16), (P, HID))
            )
            wdict[f"lnb{i}"] = np.ascontiguousarray(
                np.broadcast_to(ln_b[i].reshape(1, HID).astype(NP_F16), (P, HID))
            )

    in_maps = []
    for c in range(C):
        m = dict(wdict)
        lo, hi = c * RPC, min((c + 1) * RPC, N)
        xT = np.zeros((IN_DIM, NPC), np.float32)
        xT[:, 0 : hi - lo] = x[lo:hi].T
        m["xT"] = xT.astype(NP_F16)
        m["kvidxA"] = tconv[c]["kv_idxA"]
        m["kvidxB"] = tconv[c]["kv_idxB"]
        m["S_t"] = tconv[c]["S"]
        m["ST_t"] = tconv[c]["ST"]
        m["gatidxA"] = gat[c]["kv_idxA"]
        m["gatidxB"] = gat[c]["kv_idxB"]
        m["S_g"] = gat[c]["S"]
        m["ST_g"] = gat[c]["ST"]
        Sp = np.zeros((NB, P, N_GRAPHS), dtype=NP_F8)
        ns = hi - lo
        bb = np.arange(ns) // P
        pp = np.arange(ns) % P
        Sp[bb, pp, batch[lo:hi]] = 1.0
        m["S_p"] = np.ascontiguousarray(Sp.reshape(NB * P, N_GRAPHS))
        in_maps.append(m)

    meta = dict(
        NB=NB, NPC=NPC, NTTA=NTTA, NTTB=NTTB, NTGA=NTGA, NTGB=NTGB,
        ln_trivial=ln_trivial,
    )
    return meta, in_maps


def build_program(meta, reps=1, skip=frozenset()):
    NB = meta["NB"]
    NPC = meta["NPC"]
    NTTA, NTTB = meta["NTTA"], meta["NTTB"]
    NTGA, NTGB = meta["NTGA"], meta["NTGB"]
    ln_trivial = meta["ln_trivial"]
    HNPC = NPC // 2
    HTABN = C * HNPC
    NTT = NTTA + NTTB  # j-tiles per block
    NTG = NTGA + NTGB
    NTMAX = max(NTT, NTG)
    GATW = 256  # f16 cols: 0:128 hh as fp8 (bitcast), 128:136 a_s, rest zero

    nc = bacc.Bacc(
        "TRN2", target_bir_lowering=False, debug=False, num_devices=C,
        num_swdge_queues=4, dynamic_dma_scratch_size=14336,
    )
    _gq_counter = [0]

    def _next_gq():
        q = _gq_counter[0] % 4
        _gq_counter[0] += 1
        return q

    def di(name, shape, dt):
        return nc.dram_tensor(name, shape, dt, kind="ExternalInput")

    xT_d = di("xT", [IN_DIM, NPC], F16)
    kvidxA_d = di("kvidxA", [P, NB * NTTA * 8], I16)
    kvidxB_d = di("kvidxB", [P, NB * NTTB * 8], I16)
    St_d = di("S_t", [NB * P, NTT * P], F8)
    STt_d = di("ST_t", [NB * P, NTT * P], F8)
    gatidxA_d = di("gatidxA", [P, NB * NTGA * 8], I16)
    gatidxB_d = di("gatidxB", [P, NB * NTGB * 8], I16)
    Sg_d = di("S_g", [NB * P, NTG * P], F8)
    STg_d = di("ST_g", [NB * P, NTG * P], F8)
    Sp_d = di("S_p", [NB * P, N_GRAPHS], F8)
    invcnt_d = di("invcnt", [N_GRAPHS, 1], F32)
    win_d = di("win", [IN_DIM, HID], F16)
    bin_d = di("b_in", [1, HID], F16)
    w1_d = di("w1", [P, 2 * 2 * HID], F16)
    b1_d = di("b1", [1, 2 * HID], F16)
    w2_d = di("w2", [P, 4 * OUT_DIM], F16)
    b2_d = di("b2", [1, OUT_DIM], F16)
    wd = {}
    for t in range(2):
        wd[f"wkv{t}"] = di(f"wkv{t}", [P, 2 * 2 * HID], F16)
        wd[f"bkv{t}"] = di(f"bkv{t}", [1, 2 * HID], F16)
        wd[f"wq{t}"] = di(f"wq{t}", [P, 2 * HID], F16)
        wd[f"bq{t}"] = di(f"bq{t}", [1, HID], F16)
        wd[f"wsk{t}"] = di(f"wsk{t}", [P, 2 * HID], F16)
        wd[f"bsk{t}"] = di(f"bsk{t}", [1, HID], F16)
        wd[f"wg{t}"] = di(f"wg{t}", [P, 2 * HID], F16)
        wd[f"bg{t}"] = di(f"bg{t}", [1, HID], F16)
        wd[f"atts{t}"] = di(f"atts{t}", [P, HID], F16)
        wd[f"attd{t}"] = di(f"attd{t}", [P, HID], F16)
    if not ln_trivial:
        for i in range(4):
            wd[f"lng{i}"] = di(f"lng{i}", [P, HID], F16)
            wd[f"lnb{i}"] = di(f"lnb{i}", [P, HID], F16)

    out_d = nc.dram_tensor("out", [N_GRAPHS, OUT_DIM], F32, kind="ExternalOutput")

    h_all = nc.alloc_sbuf_tensor("h_all", [P, NB * HID], F16)
    hT_all = nc.alloc_sbuf_tensor("hT_all", [P, 2 * NPC], F8)
    # per-block dst-side projections kept on-chip (expanded to edge slots by
    # one-hot S^T matmuls -- never gathered)
    q8_all = nc.alloc_sbuf_tensor("q8_all", [P, NB * HID], F8)
    ad_all = nc.alloc_sbuf_tensor("ad_all", [P, NB * HEADS], F16)

    SQ32 = 1.0 / float(np.sqrt(HDIM))

    with tile.TileContext(nc) as tc:
        with (
            tc.tile_pool(name="wpool", bufs=1) as wp,
            tc.tile_pool(name="spool", bufs=2) as sp,
            tc.tile_pool(name="gpool", bufs=2) as gp,
            tc.tile_pool(name="psA", bufs=1, space="PSUM") as psA,
            tc.tile_pool(name="psB", bufs=1, space="PSUM") as psB,
            tc.tile_pool(name="psG", bufs=2, space="PSUM") as psG,
            tc.tile_pool(name="psQ", bufs=2, space="PSUM") as psQ,
            tc.tile_pool(name="psT", bufs=1, space="PSUM") as psT,
            tc.tile_pool(name="psC", bufs=1, space="PSUM") as psC,
            tc.tile_pool(name="dram", bufs=1, space="DRAM") as dp,
        ):
            ident = wp.tile([P, P], F16, tag="ident")
            make_identity(nc, ident[:])
            ones1 = wp.tile([1, P], F16, tag="ones1")
            nc.vector.memset(ones1[:], 1.0)
            eps_t = wp.tile([P, 1], F32, tag="eps")
            nc.vector.memset(eps_t[:], LN_EPS)

            def load_w(d, shape, tag, dt=F16):
                t = wp.tile(shape, dt, tag=tag)
                nc.sync.dma_start(t[:], d.ap())
                return t

            win_t = load_w(win_d, [IN_DIM, HID], "win")
            bin_t = load_w(bin_d, [1, HID], "b_in")
            w1_t = load_w(w1_d, [P, 2 * 2 * HID], "w1")
            b1_t = load_w(b1_d, [1, 2 * HID], "b1")
            w2_t = load_w(w2_d, [P, 4 * OUT_DIM], "w2")
            b2_t = load_w(b2_d, [1, OUT_DIM], "b2")
            wt = {}
            for t in range(2):
                for nm, sh in [
                    (f"wkv{t}", [P, 2 * 2 * HID]),
                    (f"bkv{t}", [1, 2 * HID]),
                    (f"wq{t}", [P, 2 * HID]),
                    (f"bq{t}", [1, HID]),
                    (f"wsk{t}", [P, 2 * HID]),
                    (f"bsk{t}", [1, HID]),
                    (f"wg{t}", [P, 2 * HID]),
                    (f"bg{t}", [1, HID]),
                    (f"atts{t}", [P, HID]),
                    (f"attd{t}", [P, HID]),
                ]:
                    wt[nm] = load_w(wd[nm], sh, nm)
            if not ln_trivial:
                for i in range(4):
                    wt[f"lng{i}"] = load_w(wd[f"lng{i}"], [P, HID], f"lng{i}")
                    wt[f"lnb{i}"] = load_w(wd[f"lnb{i}"], [P, HID], f"lnb{i}")
            invcnt_t = load_w(invcnt_d, [N_GRAPHS, 1], "invcnt", F32)

            def mm_dense(psum, lhsT0, lhsT1, w_tile, ncols, bias_tile):
                nc.tensor.matmul(
                    psum, lhsT=lhsT0, rhs=w_tile[:, 0:ncols], start=True, stop=False
                )
                nc.tensor.matmul(
                    psum, lhsT=lhsT1, rhs=w_tile[:, ncols : 2 * ncols],
                    start=False, stop=False,
                )
                nc.tensor.matmul(
                    psum, lhsT=ones1[:], rhs=bias_tile[:, 0:ncols],
                    start=False, stop=True,
                )

            def hT_slices(b):
                l0 = hT_all.ap()[:, 0 * NPC + b * P : 0 * NPC + (b + 1) * P]
                l1 = hT_all.ap()[:, 1 * NPC + b * P : 1 * NPC + (b + 1) * P]
                return l0, l1

            def store_hT(b):
                """h_all (f16) block b -> transposed fp8 copies in hT_all.
                Both 128-col transposes land in one PSUM tile; a single
                strided ACT copy writes both hT_all chunks."""
                hsl = h_all.ap()[:, b * HID : (b + 1) * HID]
                ptp = psT.tile([P, 2, P], F16, space="PSUM", tag="ptp")
                for f in range(2):
                    nc.tensor.transpose(
                        ptp[:, f, :], hsl[:, f * P : (f + 1) * P], ident[:]
                    )
                nc.scalar.activation(
                    hT_all.ap().rearrange("p (f c) -> p f c", f=2)[
                        :, :, b * P : (b + 1) * P
                    ],
                    ptp[:],
                    AF.Copy,
                )

            for _rep in range(reps):
                # phase 0: h0 = x @ Win + b_in
                for b in range(NB):
                    xt_b = sp.tile([IN_DIM, P], F16, tag="xT_b")
                    nc.sync.dma_start(xt_b[:], xT_d.ap()[:, b * P : (b + 1) * P])
                    ps = psA.tile([P, 2 * HID], F32, space="PSUM", tag="ps_dense")
                    nc.tensor.matmul(
                        ps[:, 0:HID], lhsT=xt_b[:],
                        rhs=win_t[:], start=True, stop=False,
                    )
                    nc.tensor.matmul(
                        ps[:, 0:HID], lhsT=ones1[:], rhs=bin_t[:], start=False,
                        stop=True,
                    )
                    nc.scalar.activation(
                        h_all.ap()[:, b * HID : (b + 1) * HID], ps[:, 0:HID], AF.Copy
                    )
                    store_hT(b)

                # ---- tables: per-layer, per-half fp8 AllGather outputs ----
                # Each AllGather half is a separate Shared tensor (single-writer
                # rule) so the second half can fly while edges of the first half
                # are being processed. GAT rows are hh-only fp8 (a_s is
                # recomputed per edge on-chip).
                kv_tabs = [
                    [
                        dp.tile(
                            [HTABN, 2 * HID], F8, tag=f"kv_tab{t}{h}_r{_rep}",
                            name=f"kv_tab{t}{h}_r{_rep}", addr_space="Shared",
                        )
                        for h in range(2)
                    ]
                    for t in range(2)
                ]
                gat_tabs = [
                    [
                        dp.tile(
                            [HTABN, GATW], F16, tag=f"gat_tab{t}{h}_r{_rep}",
                            name=f"gat_tab{t}{h}_r{_rep}", addr_space="Shared",
                        )
                        for h in range(2)
                    ]
                    for t in range(2)
                ]
                kv_bnc = dp.tile([NPC, 2 * HID], F8, tag=f"kv_bnc_r{_rep}")
                gat_bnc = dp.tile([NPC, GATW], F16, tag=f"gat_bnc_r{_rep}")
                # zero the unread tail columns once so the AllGather input is
                # fully initialized
                zpad = wp.tile([P, GATW], F16, tag="zpad")
                nc.vector.memset(zpad[:], 0.0)
                for b in range(NB):
                    nc.sync.dma_start(
                        gat_bnc[b * P : (b + 1) * P, HID // 2 + HEADS : GATW],
                        zpad[:, 0 : GATW - HID // 2 - HEADS],
                    )
                pool_in = dp.tile([N_GRAPHS, HID], F32, tag=f"pool_in_r{_rep}")
                pool_out = dp.tile([N_GRAPHS, HID], F32, tag=f"pool_out_r{_rep}")

                psum_pool = psC.tile([N_GRAPHS, HID], F32, space="PSUM", tag="ps_pool")
                agg_sb_q = sp.tile([P, 4, HID + HEADS], F16, tag="agg_sb")
                skp_sb_q = sp.tile([P, 4, HID], F16, tag="skp_sb")
                quad = {"agg": agg_sb_q, "skp": skp_sb_q, "blocks": []}

                def dense_pair(layer, bp):
                    """Dense projections for blocks (bp, bp+1) of `layer` into
                    the bounce buffers / on-chip q8/ad tables."""
                    if "dense" in skip:
                        return
                    is_t = layer % 2 == 0
                    t = layer // 2
                    nb2 = min(2, NB - bp)
                    prow = (
                        lambda tab, c0, c1: tab[bp * P : (bp + nb2) * P, c0:c1]
                        .rearrange("(i p) f -> p i f", i=nb2)
                    )
                    if is_t:
                        kv8p = sp.tile([P, 2, 2 * HID], F8, tag="kv8")
                    else:
                        hh8p = sp.tile([P, 2, HID], F8, tag="hh8")
                        as16p = sp.tile([P, 2, HEADS], F16, tag="as16")
                    for i in range(nb2):
                        b = bp + i
                        l0, l1 = hT_slices(b)
                        if is_t:
                            ps = psA.tile(
                                [P, 2 * HID], F32, space="PSUM", tag="ps_dense"
                            )
                            mm_dense(
                                ps[:], l0, l1, wt[f"wkv{t}"], 2 * HID, wt[f"bkv{t}"]
                            )
                            nc.scalar.activation(kv8p[:, i, :], ps[:], AF.Copy)
                            ps2 = psB.tile([P, 2 * HID], F32, space="PSUM", tag="ps_b")
                            mm_dense(
                                ps2[:, 0:HID], l0, l1, wt[f"wq{t}"], HID, wt[f"bq{t}"]
                            )
                            nc.scalar.activation(
                                q8_all.ap()[:, b * HID : (b + 1) * HID],
                                ps2[:, 0:HID], AF.Copy,
                            )
                        else:
                            ps = psA.tile(
                                [P, 2 * HID], F32, space="PSUM", tag="ps_dense"
                            )
                            mm_dense(
                                ps[:, 0:HID], l0, l1, wt[f"wg{t}"], HID, wt[f"bg{t}"]
                            )
                            nc.scalar.activation(hh8p[:, i, :], ps[:, 0:HID], AF.Copy)
                            for which, wnm in ((0, f"atts{t}"), (1, f"attd{t}")):
                                proda = sp.tile([P, HID], F16, tag="prodA")
                                nc.vector.tensor_tensor(
                                    out=proda[:], in0=ps[:, 0:HID], in1=wt[wnm][:],
                                    op=mybir.AluOpType.mult,
                                )
                                asum = sp.tile([P, HEADS], F32, tag="asum")
                                nc.vector.tensor_reduce(
                                    out=asum[:],
                                    in_=proda[:].rearrange(
                                        "p (h w) -> p h w", h=HEADS
                                    ),
                                    axis=mybir.AxisListType.X,
                                    op=mybir.AluOpType.add,
                                )
                                dst = (
                                    as16p[:, i, :]
                                    if which == 0
                                    else ad_all.ap()[:, b * HEADS : (b + 1) * HEADS]
                                )
                                nc.scalar.activation(dst, asum[:], AF.Copy)
                    if is_t:
                        nc.sync.dma_start(prow(kv_bnc, 0, 2 * HID), kv8p[:, 0:nb2, :])
                    else:
                        nc.sync.dma_start(
                            prow(gat_bnc, 0, HID // 2), hh8p[:, 0:nb2, :].bitcast(F16)
                        )
                        nc.sync.dma_start(
                            prow(gat_bnc, HID // 2, HID // 2 + HEADS),
                            as16p[:, 0:nb2, :],
                        )

                def emit_ag(layer, half):
                    """AllGather one half of this layer's table."""
                    if "ag" in skip:
                        return
                    is_t = layer % 2 == 0
                    t = layer // 2
                    rows = slice(half * HNPC, (half + 1) * HNPC)
                    if is_t:
                        nc.gpsimd.collective_compute(
                            "AllGather",
                            mybir.AluOpType.bypass,
                            replica_groups=[list(range(C))],
                            ins=[kv_bnc[rows, :]],
                            outs=[kv_tabs[t][half][:]],
                        )
                    else:
                        nc.gpsimd.collective_compute(
                            "AllGather",
                            mybir.AluOpType.bypass,
                            replica_groups=[list(range(C))],
                            ins=[gat_bnc[rows, :]],
                            outs=[gat_tabs[t][half][:]],
                        )

                def edge_pair(layer, bp):
                    if "edge" in skip:
                        return
                    do_edvec = "edvec" not in skip
                    do_scatter = "scatter" not in skip
                    do_norm = do_scatter and "norm" not in skip
                    is_t = layer % 2 == 0
                    t = layer // 2
                    nb2 = min(2, NB - bp)
                    NTa = NTTA if is_t else NTGA
                    NTb = NTTB if is_t else NTGB
                    NT = NTa + NTb
                    TT = nb2 * NT
                    N2a = nb2 * NTa
                    islA = slice(bp * NTa * 8, (bp + nb2) * NTa * 8)
                    islB = slice(bp * NTb * 8, (bp + nb2) * NTb * 8)
                    tabs = kv_tabs[t] if is_t else gat_tabs[t]
                    S_d = St_d if is_t else Sg_d
                    ST_d = STt_d if is_t else STg_d

                    def tmap(tg):
                        """g_kv tile index -> (block-in-pair, block-local tile)."""
                        if tg < N2a:
                            return tg // NTa, tg % NTa
                        tg -= N2a
                        return tg // NTb, NTa + tg % NTb

                    if is_t:
                        g_kv = gp.tile([P, 2 * NT, 2 * HID], F8, tag="g_big1")
                        ROWW = 2 * HID
                        idxA_d, idxB_d = kvidxA_d, kvidxB_d
                    else:
                        g_kv = gp.tile([P, 2 * NT, GATW], F16, tag="g_big0")
                        ROWW = GATW
                        idxA_d, idxB_d = gatidxA_d, gatidxB_d
                    idxA_t = gp.tile(
                        [P, 2 * max(NTTA, NTGA) * 8], I16, tag="idxA"
                    )
                    idxB_t = gp.tile(
                        [P, 2 * max(NTTB, NTGB) * 8], I16, tag="idxB"
                    )
                    nc.sync.dma_start(
                        idxA_t[:, 0 : nb2 * NTa * 8], idxA_d.ap()[:, islA]
                    )
                    nc.sync.dma_start(
                        idxB_t[:, 0 : nb2 * NTb * 8], idxB_d.ap()[:, islB]
                    )
                    # tiles per gather call: must fit the SWDGE ring
                    # (896 descs); 7 for GAT avoids a 2-tile runt call
                    GCH = 6 if is_t else 7
                    if "gather" not in skip:
                        for t0 in range(0, N2a, GCH):
                            tn = min(GCH, N2a - t0)
                            nc.gpsimd.dma_gather(
                                g_kv[:, t0 : t0 + tn, :], tabs[0][:],
                                idxA_t[:, t0 * 8 : (t0 + tn) * 8],
                                tn * P, tn * P, ROWW, single_packet=False,
                                queue_num=_next_gq(),
                            )
                        nbt = nb2 * NTb
                        for t0 in range(0, nbt, GCH):
                            tn = min(GCH, nbt - t0)
                            nc.gpsimd.dma_gather(
                                g_kv[:, N2a + t0 : N2a + t0 + tn, :], tabs[1][:],
                                idxB_t[:, t0 * 8 : (t0 + tn) * 8],
                                tn * P, tn * P, ROWW, single_packet=False,
                                queue_num=_next_gq(),
                            )
                    if is_t:
                        vpart = g_kv[:, 0:TT, HID : 2 * HID]
                    else:
                        vpart = g_kv[:, 0:TT, 0 : HID // 2].bitcast(F8)

                    ST_sb = []
                    for i in range(nb2):
                        b = bp + i
                        stt = gp.tile([P, NTMAX * P], F8, tag=f"ST{i}")
                        nc.sync.dma_start(
                            stt[:, 0 : NT * P], ST_d.ap()[b * P : (b + 1) * P, :]
                        )
                        ST_sb.append(stt)

                    rhs = gp.tile([P, 2 * NTMAX, HID + HEADS], F16, tag="rhs")
                    red = gp.tile([P, 2 * NTMAX * HEADS], F16, tag="red")
                    expdst = rhs[:, 0:TT, HID : HID + HEADS]
                    if not do_edvec:
                        pass
                    elif is_t:
                        # q[dst] expanded per edge slot: psq[:, g, :] =
                        # ST_tile^T @ q8_block, then k * q product on DVE.
                        for pt in range((TT + 1) // 2):
                            ng = min(2, TT - 2 * pt)
                            psq = psQ.tile([P, 2, HID], F32, space="PSUM", tag="psq")
                            for g in range(ng):
                                i, tl = tmap(2 * pt + g)
                                nc.tensor.matmul(
                                    psq[:, g, :],
                                    lhsT=ST_sb[i][:, tl * P : (tl + 1) * P],
                                    rhs=q8_all.ap()[
                                        :, (bp + i) * HID : (bp + i + 1) * HID
                                    ],
                                    start=True, stop=True,
                                )
                            nc.vector.tensor_tensor(
                                out=rhs[:, 2 * pt : 2 * pt + ng, 0:HID],
                                in0=g_kv[:, 2 * pt : 2 * pt + ng, 0:HID],
                                in1=psq[:, 0:ng, :],
                                op=mybir.AluOpType.mult,
                            )
                        with nc.allow_low_precision(reason="f16 logits"):
                            nc.vector.tensor_reduce(
                                out=red[:, 0 : TT * HEADS],
                                in_=rhs[:, 0:TT, 0:HID].rearrange(
                                    "p t (h w) -> p t h w", h=HEADS
                                ),
                                axis=mybir.AxisListType.X,
                                op=mybir.AluOpType.add,
                            )
                        nc.scalar.activation(
                            expdst,
                            red[:, 0 : TT * HEADS].rearrange(
                                "p (t h) -> p t h", h=HEADS
                            ),
                            AF.Exp,
                            scale=SQ32,
                        )
                    else:
                        # a_d[dst] expanded per edge slot into PSUM (8-col
                        # one-hot matmuls); a_s recomputed from gathered hh.
                        psq = psQ.tile([P, 2, HID], F32, space="PSUM", tag="psq")
                        psad = (
                            psq[:]
                            .rearrange("p a b -> p (a b)")[:, 0 : TT * HEADS]
                            .rearrange("p (t h) -> p t h", h=HEADS)
                        )
                        for tg in range(TT):
                            i, tl = tmap(tg)
                            nc.tensor.matmul(
                                psad[:, tg, :],
                                lhsT=ST_sb[i][:, tl * P : (tl + 1) * P],
                                rhs=ad_all.ap()[
                                    :, (bp + i) * HEADS : (bp + i + 1) * HEADS
                                ],
                                start=True, stop=True,
                            )
                        esum = gp.tile([P, 2 * NTMAX * HEADS], F16, tag="esum")
                        nc.vector.tensor_tensor(
                            out=esum[:, 0 : TT * HEADS].rearrange(
                                "p (t h) -> p t h", h=HEADS
                            ),
                            in0=g_kv[:, 0:TT, HID // 2 : HID // 2 + HEADS],
                            in1=psad[:, 0:TT, :],
                            op=mybir.AluOpType.add,
                        )
                        # leaky_relu(x, 0.2) = 0.6x + 0.4|x| (expdst doubles
                        # as |x| scratch; Exp overwrites it right after)
                        esum3 = esum[:, 0 : TT * HEADS].rearrange(
                            "p (t h) -> p t h", h=HEADS
                        )
                        nc.scalar.activation(expdst, esum3, AF.Abs, scale=0.4)
                        nc.vector.scalar_tensor_tensor(
                            out=red[:, 0 : TT * HEADS].rearrange(
                                "p (t h) -> p t h", h=HEADS
                            ),
                            in0=esum3,
                            scalar=0.6,
                            in1=expdst,
                            op0=mybir.AluOpType.mult,
                            op1=mybir.AluOpType.add,
                        )
                        nc.scalar.activation(
                            expdst,
                            red[:, 0 : TT * HEADS].rearrange(
                                "p (t h) -> p t h", h=HEADS
                            ),
                            AF.Exp,
                        )
                    if do_edvec:
                        nc.vector.tensor_tensor(
                            out=rhs[:, 0:TT, 0:HID].rearrange(
                                "p t (h w) -> p t h w", h=HEADS
                            ),
                            in0=vpart.rearrange("p t (h w) -> p t h w", h=HEADS),
                            in1=expdst.to_broadcast([P, TT, HEADS, HDIM]),
                            op=mybir.AluOpType.mult,
                        )
                    # per-block scatter + skip matmuls
                    aggs = []
                    for i in range(nb2 if do_scatter else 0):
                        b = bp + i
                        # reuse the ST tile: its one-hot data has been fully
                        # consumed by the psq/psad matmuls by this point
                        S_sb = ST_sb[i]
                        nc.sync.dma_start(
                            S_sb[:, 0 : NT * P], S_d.ap()[b * P : (b + 1) * P, :]
                        )
                        ps_agg = psG.tile(
                            [P, HID + HEADS], F32, space="PSUM", tag="ps_agg"
                        )
                        for jj in range(NTa):
                            nc.tensor.matmul(
                                ps_agg[:],
                                lhsT=S_sb[:, jj * P : (jj + 1) * P],
                                rhs=rhs[:, i * NTa + jj, :],
                                start=(jj == 0),
                                stop=False,
                            )
                        for jj in range(NTb):
                            nc.tensor.matmul(
                                ps_agg[:],
                                lhsT=S_sb[:, (NTa + jj) * P : (NTa + jj + 1) * P],
                                rhs=rhs[:, N2a + i * NTb + jj, :],
                                start=False,
                                stop=(jj == NTb - 1),
                            )
                        l0, l1 = hT_slices(b)
                        ps_skip = psB.tile([P, 2 * HID], F32, space="PSUM", tag="ps_b")
                        if is_t:
                            mm_dense(
                                ps_skip[:, 0:HID], l0, l1, wt[f"wsk{t}"], HID,
                                wt[f"bsk{t}"],
                            )
                        else:
                            nc.tensor.matmul(
                                ps_skip[:, 0:HID], lhsT=ones1[:], rhs=wt[f"bg{t}"][:],
                                start=True, stop=True,
                            )
                        aggs.append((ps_agg, ps_skip))

                    # stage PSUM accumulators to SBUF on ACT; the DVE
                    # normalize chain runs once per TWO pairs (4 blocks) in
                    # norm_quad so its op count halves.
                    if not do_norm:
                        return
                    q = (bp // 2) % 2  # slot within the quad staging buffer
                    agg_sb = quad["agg"]
                    skp_sb = quad["skp"]
                    for i in range(nb2):
                        ps_agg, ps_skip = aggs[i]
                        nc.scalar.activation(
                            agg_sb[:, 2 * q + i, :], ps_agg[:], AF.Copy
                        )
                        nc.scalar.activation(
                            skp_sb[:, 2 * q + i, :], ps_skip[:, 0:HID], AF.Copy
                        )
                    quad["blocks"] += list(range(bp, bp + nb2))

                def norm_quad(layer):
                    """Normalize + LN + relu for the staged quad of blocks."""
                    is_t = layer % 2 == 0
                    blocks = quad["blocks"]
                    nb2 = len(blocks)
                    if nb2 == 0:
                        return
                    quad["blocks"] = []
                    agg_sb = quad["agg"]
                    skp_sb = quad["skp"]
                    bp = blocks[0]
                    t2p = sp.tile([P, 4, HID], F16, tag="t2p")
                    den = sp.tile([P, 4, HEADS], F32, tag="den")
                    nc.vector.tensor_scalar(
                        out=den[:, 0:nb2, :],
                        in0=agg_sb[:, 0:nb2, HID : HID + HEADS],
                        scalar1=1e-16, scalar2=None, op0=mybir.AluOpType.add,
                    )
                    rec = sp.tile([P, 4, HEADS], F32, tag="rec")
                    nc.vector.reciprocal(rec[:, 0:nb2, :], den[:, 0:nb2, :])
                    nc.vector.tensor_tensor(
                        out=t2p[:, 0:nb2, :].rearrange(
                            "p i (h w) -> p i h w", h=HEADS
                        ),
                        in0=agg_sb[:, 0:nb2, 0:HID].rearrange(
                            "p i (h w) -> p i h w", h=HEADS
                        ),
                        in1=rec[:, 0:nb2, :].to_broadcast([P, nb2, HEADS, HDIM]),
                        op=mybir.AluOpType.mult,
                    )
                    nc.vector.tensor_tensor(
                        out=t2p[:, 0:nb2, :], in0=t2p[:, 0:nb2, :],
                        in1=skp_sb[:, 0:nb2, :],
                        op=mybir.AluOpType.add,
                    )
                    t2v = t2p[:, 0:nb2, :]
                    if not is_t:
                        nc.vector.tensor_tensor(
                            out=t2v, in0=t2v,
                            in1=h_all.ap()[:, bp * HID : (bp + nb2) * HID].rearrange(
                                "p (i f) -> p i f", i=nb2
                            ),
                            op=mybir.AluOpType.add,
                        )
                    assert blocks == list(range(bp, bp + nb2))
                    mu = sp.tile([P, 4], F32, tag="mu")
                    nc.vector.tensor_reduce(
                        out=mu[:, 0:nb2], in_=t2v, axis=mybir.AxisListType.X,
                        op=mybir.AluOpType.add,
                    )
                    nc.vector.tensor_scalar(
                        out=mu[:, 0:nb2], in0=mu[:, 0:nb2], scalar1=1.0 / HID,
                        scalar2=None, op0=mybir.AluOpType.mult,
                    )
                    nc.vector.tensor_tensor(
                        out=t2v, in0=t2v,
                        in1=mu[:, 0:nb2].to_broadcast([P, nb2, HID]),
                        op=mybir.AluOpType.subtract,
                    )
                    sq = sp.tile([P, 4, HID], F16, tag="sq")
                    nc.scalar.activation(sq[:, 0:nb2, :], t2v, AF.Square)
                    s2 = sp.tile([P, 4], F32, tag="s2")
                    nc.vector.tensor_reduce(
                        out=s2[:, 0:nb2], in_=sq[:, 0:nb2, :],
                        axis=mybir.AxisListType.X, op=mybir.AluOpType.add,
                    )
                    # rsqrt via exp(-0.5*ln(x)): Sqrt lives in an ACT
                    # table set without Exp, so Sqrt/Exp alternation would
                    # reload the ACT function table each pair.
                    sd = sp.tile([P, 4], F32, tag="sd")
                    nc.scalar.activation(
                        sd[:, 0:nb2], s2[:, 0:nb2], AF.Ln, scale=1.0 / HID,
                        bias=eps_t[:, 0:1],
                    )
                    rs = sp.tile([P, 4], F32, tag="rs")
                    nc.scalar.activation(
                        rs[:, 0:nb2], sd[:, 0:nb2], AF.Exp, scale=-0.5
                    )
                    nc.vector.tensor_tensor(
                        out=t2v, in0=t2v,
                        in1=rs[:, 0:nb2].to_broadcast([P, nb2, HID]),
                        op=mybir.AluOpType.mult,
                    )
                    if not ln_trivial:
                        nc.vector.tensor_tensor(
                            out=t2v, in0=t2v,
                            in1=wt[f"lng{layer}"][:]
                            .rearrange("p (o f) -> p o f", o=1)
                            .to_broadcast([P, nb2, HID]),
                            op=mybir.AluOpType.mult,
                        )
                        nc.vector.tensor_tensor(
                            out=t2v, in0=t2v,
                            in1=wt[f"lnb{layer}"][:]
                            .rearrange("p (o f) -> p o f", o=1)
                            .to_broadcast([P, nb2, HID]),
                            op=mybir.AluOpType.add,
                        )
                    hdst = h_all.ap()[:, bp * HID : (bp + nb2) * HID]
                    nc.vector.tensor_scalar(
                        out=hdst.rearrange("p (i f) -> p i f", i=nb2), in0=t2v,
                        scalar1=0.0, scalar2=None, op0=mybir.AluOpType.max,
                    )
                    for i in range(nb2):
                        b = bp + i
                        if "sth" not in skip:
                            store_hT(b)
                        if layer == 3:
                            spt = sp.tile([P, N_GRAPHS], F8, tag="Sp_b")
                            nc.sync.dma_start(
                                spt[:], Sp_d.ap()[b * P : (b + 1) * P, :]
                            )
                            nc.tensor.matmul(
                                psum_pool[:],
                                lhsT=spt[:],
                                rhs=h_all.ap()[:, b * HID : (b + 1) * HID],
                                start=(b == 0),
                                stop=(b == NB - 1),
                            )

                # layer 0 dense phase + its chunked AllGathers
                for bp in range(0, NB, 2):
                    dense_pair(0, bp)
                    if bp == NB // 2 - 2:
                        emit_ag(0, 0)
                emit_ag(0, 1)

                # main loop: edge phase of layer L interleaved with dense phase
                # of layer L+1; each half-AllGather is emitted as soon as its
                # bounce rows are complete so it overlaps remaining edge/dense
                # work.
                for layer in range(4):
                    for bp in range(0, NB, 2):
                        edge_pair(layer, bp)
                        if (bp // 2) % 2 == 1 or bp + 2 >= NB:
                            norm_quad(layer)
                            agg_sb_q = sp.tile(
                                [P, 4, HID + HEADS], F16, tag="agg_sb"
                            )
                            skp_sb_q = sp.tile([P, 4, HID], F16, tag="skp_sb")
                            quad["agg"] = agg_sb_q
                            quad["skp"] = skp_sb_q
                            if layer < 3:
                                for bq in range(max(0, bp - 2), bp + 2, 2):
                                    dense_pair(layer + 1, bq)
                                    if bq == NB // 2 - 2:
                                        emit_ag(layer + 1, 0)
                                    elif bq == NB - 2:
                                        emit_ag(layer + 1, 1)

                # ---- pool + MLP ----
                if skip & {"edge", "scatter", "norm"}:  # ablation: pool unwritten
                    nc.tensor.matmul(
                        psum_pool[:], lhsT=ident[:], rhs=h_all.ap()[:, 0:HID],
                        start=True, stop=True,
                    )
                pool_sb = sp.tile([N_GRAPHS, HID], F32, tag="pool_sb")
                nc.scalar.activation(pool_sb[:], psum_pool[:], AF.Copy)
                nc.sync.dma_start(pool_in[:], pool_sb[:])
                if "ar" not in skip:
                    nc.gpsimd.collective_compute(
                        "AllReduce",
                        mybir.AluOpType.add,
                        replica_groups=[list(range(C))],
                        ins=[pool_in.opt()],
                        outs=[pool_out.opt()],
                    )
                sums = sp.tile([N_GRAPHS, HID], F32, tag="sums")
                nc.sync.dma_start(
                    sums[:], pool_in[:] if "ar" in skip else pool_out[:]
                )
                pooled = sp.tile([N_GRAPHS, HID], F32, tag="pooled")
                nc.vector.tensor_scalar(
                    out=pooled[:], in0=sums[:], scalar1=invcnt_t[:, 0:1],
                    scalar2=None, op0=mybir.AluOpType.mult,
                )
                p16 = sp.tile([N_GRAPHS, HID], F16, tag="p16")
                nc.scalar.activation(p16[:], pooled[:], AF.Copy)
                pT = sp.tile([P, 2 * N_GRAPHS], F16, tag="pT")
                for f in range(2):
                    ptp = psT.tile([P, P], F16, space="PSUM", tag="ptp")
                    nc.tensor.transpose(ptp[:], p16[:, f * P : (f + 1) * P], ident[:])
                    nc.scalar.activation(
                        pT[:, f * N_GRAPHS : (f + 1) * N_GRAPHS], ptp[:], AF.Copy
                    )
                ps1 = psA.tile([P, 2 * HID], F32, space="PSUM", tag="ps_dense")
                nc.tensor.matmul(
                    ps1[:], lhsT=pT[:, 0:N_GRAPHS], rhs=w1_t[:, 0 : 2 * HID],
                    start=True, stop=False,
                )
                nc.tensor.matmul(
                    ps1[:], lhsT=pT[:, N_GRAPHS : 2 * N_GRAPHS],
                    rhs=w1_t[:, 2 * HID : 4 * HID], start=False, stop=False,
                )
                nc.tensor.matmul(
                    ps1[:], lhsT=ones1[:], rhs=b1_t[:], start=False, stop=True
                )
                h1 = sp.tile([N_GRAPHS, 2 * HID], F16, tag="h1")
                nc.scalar.activation(h1[:], ps1[:], AF.Relu)
                h1T = sp.tile([P, 4 * N_GRAPHS], F16, tag="h1T")
                for f in range(4):
                    ptp = psT.tile([P, P], F16, space="PSUM", tag="ptp")
                    nc.tensor.transpose(ptp[:], h1[:, f * P : (f + 1) * P], ident[:])
                    nc.scalar.activation(
                        h1T[:, f * N_GRAPHS : (f + 1) * N_GRAPHS], ptp[:], AF.Copy
                    )
                ps2 = psB.tile([P, 2 * HID], F32, space="PSUM", tag="ps_b")
                for f in range(4):
                    nc.tensor.matmul(
                        ps2[:, 0:OUT_DIM],
                        lhsT=h1T[:, f * N_GRAPHS : (f + 1) * N_GRAPHS],
                        rhs=w2_t[:, f * OUT_DIM : (f + 1) * OUT_DIM],
                        start=(f == 0),
                        stop=False,
                    )
                nc.tensor.matmul(
                    ps2[:, 0:OUT_DIM], lhsT=ones1[:], rhs=b2_t[:], start=False,
                    stop=True,
                )
                out_sb = sp.tile([N_GRAPHS, OUT_DIM], F32, tag="out_sb")
                nc.scalar.activation(out_sb[:], ps2[:, 0:OUT_DIM], AF.Copy)
                nc.sync.dma_start(out_d.ap(), out_sb[:])

    nc.compile()
    return nc


_CACHE = {}


def kernel(**inputs):
    meta, in_maps = host_prep(inputs)
    key = tuple(sorted(meta.items()))
    if key not in _CACHE:
        _CACHE[key] = build_program(meta)
    nc = _CACHE[key]
    res = bass_utils.run_bass_kernel_spmd(nc, in_maps, core_ids=list(range(C)))
    return np.asarray(res.results[0]["out"], np.float32)
